# revision 20
# baseline (speedup 1.0000x reference)
"""Conv2d 3x3 (stride 1, pad 1) as implicit GEMM on 8 Trainium2 NeuronCores.

Problem: x [32,128,56,56] f32, weight [256,128,3,3] f32, bias [256] f32
         -> out [32,256,56,56] f32.

Sharding: data-parallel over batch. Each of the 8 cores gets 4 images;
weight/bias are replicated. No collectives; outputs are concatenated on host.

Per-core kernel (implicit GEMM, bf16 operands, fp32 PSUM accumulate):
  - x is host-padded to bf16 [4,128,58,64]: 1px conv halo + row pitch padded
    56->64 so each row slice is 128B (16B-line aligned) in SBUF.
  - weight is host-rearranged to bf16 [128, 9, 256] (in-ch partitions, 3x3
    taps, out-ch free) so lhsT slices need no on-device transpose.
  - For each image, out-channel group g (2 groups of 128) and band of 8
    output rows (7 bands): accumulate 9 matmuls (one per tap) into a
    [128, 448] PSUM tile: psum += W[:, ki, g*128:...].T @ xpad[:, rows+kh, kw:kw+56]
  - bias-add + PSUM->SBUF(bf16) on scalar/vector engines alternating,
    then DMA to DRAM (bf16). Host converts the gathered output to f32.

Measured cadence ladder at N=448 (trn2 NTFF/perfetto):
  - fp32r: 210ns/MM - LDWEIGHTS-bound (fp32 LDW 187ns + ~20ns handover
    exceeds the 186.7ns stream; fp32r cannot use standalone ldweights).
  - bf16 unpadded rows: 230ns/MM (LDW 116ns hidden; +43ns suspected from
    the 8-row rhs AP blocks at 116B stride crossing 16B SBUF lines).
  - bf16 with 128B-aligned rows: this kernel. Target ~190ns/MM.
  Per-matmul tick-sem increments are stripped down to group-final matmuls
  (26.3ns/inc measured, though removing them alone did not change cadence).

Other head/tail measures:
  - Warm-up matmuls run on a memset tile (no DMA dependency) so the PE's
    HAM clock-gate (1.2 -> 2.4 GHz, fires ~2 activity windows after PE
    work starts) warms while the first input chunk is in flight.
  - DMA triggers cost ~650ns each on their (sync/scalar) queue and Tile
    rotates ~10 completion-sem slots, so the head issues exactly 10 DMAs:
    7 image-0 chunks on sync, w-group0 / bias / w-group1 bundles on
    scalar. Later images trickle in coarser chunks.
"""

import numpy as np
import ml_dtypes

import concourse.bacc as bacc
import concourse.mybir as mybir
import concourse.tile as tile
from concourse.bass_utils import run_bass_kernel_spmd

N_CORES = 8
B, C_IN, H, W = 32, 128, 56, 56
C_OUT = 256
KH = KW = 3
B_LOC = B // N_CORES          # 4 images per core
HP = H + 2                    # 58 padded rows (conv halo)
WP = W + 2                    # 58 valid padded cols
WPAD = 64                     # row pitch in SBUF: 64 bf16 = 128B, 16B-aligned
ROWS = 8                      # output rows per matmul
NCHUNK = H // ROWS            # 7 bands
NFREE = ROWS * W              # 448 = matmul free dim (fits one PSUM bank)
NGRP = C_OUT // 128           # 2 out-channel groups

MM_DT = mybir.dt.bfloat16
OUT_DT = mybir.dt.bfloat16
BF16 = ml_dtypes.bfloat16

N_WARM = 7                    # memset-fed warm-up matmuls (N=448 @ full duty)


def _strip_mm_tick_updates(nc):
    """Remove the Tile tick-sem increment from non-stop matmuls.

    Tile attaches a `sem-inc @complete` to EVERY matmul so consumers can
    wait "first v matmuls done". Matmuls complete in queue order, so it is
    sufficient for only accumulation-group-final (stop=True) matmuls to
    increment, with every wait value remapped from "v matmuls" to "k
    stop-matmuls". All Tile-emitted waits on the tick sem land on group
    boundaries (asserted below), because the only cross-engine consumers
    of matmul completion are whole-PSUM-tile readers.
    """
    import concourse.mybir as mybir
    from collections import Counter

    mms = []
    others = []
    for f in nc.m.functions:
        for blk in f.blocks:
            for inst in blk.instructions:
                if isinstance(inst, mybir.InstMatmult):
                    mms.append(inst)
                else:
                    others.append(inst)
    ids = Counter()
    for m in mms:
        si = m.sync_info
        for u in si.on_update if si else []:
            if u.update_mode == "sem-inc":
                ids[u.id] += 1
    if not ids:
        return
    tick, cnt = ids.most_common(1)[0]
    assert cnt == len(mms), (tick, cnt, len(mms))

    kept = [bool(m.stop_tensor_calc) for m in mms]
    prefix = [0]
    for k in kept:
        prefix.append(prefix[-1] + (1 if k else 0))

    def remap(v):
        assert 0 <= v <= len(mms), v
        # wait must land on a stop-matmul boundary, else ordering is lost
        assert v == 0 or kept[v - 1], f"tick wait {v} not at a group boundary"
        return prefix[v]

    for inst in mms + others:
        si = getattr(inst, "sync_info", None)
        if not si:
            continue
        for w in si.on_wait:
            if w.sync_type == "semaphore" and w.id == tick:
                assert w.wait_mode == "sem-ge-imm", w
                w.wait_value = remap(w.wait_value)
    for m, k in zip(mms, kept):
        if not k:
            si = m.sync_info
            si.on_update = [
                u
                for u in si.on_update
                if not (u.sync_type == "semaphore" and u.id == tick)
            ]


def _build():
    nc = bacc.Bacc(None, target_bir_lowering=False)
    xp = nc.dram_tensor("xp", [B_LOC, C_IN, HP, WPAD], MM_DT, kind="ExternalInput")
    wt = nc.dram_tensor("wt", [C_IN, KH * KW, C_OUT], MM_DT, kind="ExternalInput")
    bz = nc.dram_tensor("bz", [128, NGRP], mybir.dt.float32, kind="ExternalInput")
    out = nc.dram_tensor(
        "out", [B_LOC, NGRP, 128, H * W], OUT_DT, kind="ExternalOutput"
    )

    with tile.TileContext(nc) as tc:
        with (
            tc.tile_pool(name="const", bufs=1) as cpool,
            tc.tile_pool(name="xin", bufs=B_LOC) as xpool,
            tc.tile_pool(name="oout", bufs=8) as opool,
            tc.tile_pool(name="psum", bufs=1, space="PSUM") as pspool,
        ):
            # PE warm-up with no DMA dependency: memset a tile, then issue
            # full-duty N=448 matmuls so the HAM clock-gate warms while the
            # first input chunk DMA is in flight.
            wu = cpool.tile([128, NFREE], MM_DT)
            nc.gpsimd.memset(wu[:], 0.0)
            wu_ps = pspool.tile([128, NFREE], mybir.dt.float32, tag="warm", bufs=1)
            for _ in range(N_WARM):
                nc.tensor.matmul(
                    wu_ps[:], wu[:, 0:128], wu[:], start=True, stop=True
                )

            w_tile = cpool.tile([C_IN, KH * KW, C_OUT], MM_DT)
            b_tile = cpool.tile([128, NGRP], mybir.dt.float32)
            x_tiles = [
                xpool.tile([C_IN, HP, WPAD], MM_DT, name=f"x_img{b}", tag="ximg")
                for b in range(B_LOC)
            ]

            # chunk rc of image b: band-aligned row ranges. Band rc needs
            # padded rows [rc*ROWS, rc*ROWS+ROWS+2); chunk 0 covers rows
            # 0..9, chunk rc>=1 adds rows rc*ROWS+2 .. rc*ROWS+9.
            def load_chunk(b, rc):
                lo = 0 if rc == 0 else rc * ROWS + 2
                hi = rc * ROWS + ROWS + 2
                nc.sync.dma_start(x_tiles[b][:, lo:hi], xp[b, :, lo:hi])

            # Head DMAs split across the two HWDGE trigger queues. Group-0
            # weights land as tap-triplets so band 0's first matmul is gated
            # only by chunk0 + taps 0-2, not the whole weight tensor.
            load_chunk(0, 0)
            nc.scalar.dma_start(w_tile[:, 0:3, 0:128], wt[:, 0:3, 0:128])
            load_chunk(0, 1)
            nc.scalar.dma_start(w_tile[:, 3:6, 0:128], wt[:, 3:6, 0:128])
            load_chunk(0, 2)
            nc.scalar.dma_start(b_tile[:], bz[:])
            load_chunk(0, 3)
            nc.scalar.dma_start(w_tile[:, 6:9, 0:128], wt[:, 6:9, 0:128])
            load_chunk(0, 4)
            nc.scalar.dma_start(w_tile[:, :, 128:256], wt[:, :, 128:256])
            load_chunk(0, 5)
            load_chunk(0, 6)

            # Bands per image-group: 7x8 rows, except the very last group
            # ends with two 4-row bands so the tail's eviction + final DMA
            # chain after the last matmul is half as long.
            full_bands = [(rc * ROWS, ROWS) for rc in range(NCHUNK)]
            tail_bands = full_bands[:-1] + [(48, 4), (52, 4)]
            for b in range(B_LOC):
                for g in range(NGRP):
                    last = b == B_LOC - 1 and g == NGRP - 1
                    bands = tail_bands if last else full_bands
                    for rc, (r0, nrows) in enumerate(bands):
                        # trickle next image's chunks during the g=0 pass so
                        # prefetch doesn't starve this image's output DMAs
                        if g == 0 and b + 1 < B_LOC and rc < NCHUNK:
                            load_chunk(b + 1, rc)
                        nf = nrows * W
                        ps = pspool.tile(
                            [128, nf], mybir.dt.float32, tag="ps", bufs=7
                        )
                        for ki in range(KH * KW):
                            kh, kw = divmod(ki, KW)
                            nc.tensor.matmul(
                                ps[:],
                                w_tile[:, ki, g * 128 : (g + 1) * 128],
                                x_tiles[b][
                                    :,
                                    r0 + kh : r0 + kh + nrows,
                                    kw : kw + W,
                                ],
                                start=(ki == 0),
                                stop=(ki == KH * KW - 1),
                            )
                        o_tile = opool.tile(
                            [128, nf],
                            OUT_DT,
                            name=f"o_{b}_{g}_{rc}",
                            tag="ot",
                        )
                        # alternate eviction engine: scalar and vector can
                        # read PSUM concurrently (different banks)
                        if rc % 2 == 0:
                            nc.scalar.activation(
                                o_tile[:],
                                ps[:],
                                mybir.ActivationFunctionType.Identity,
                                bias=b_tile[:, g : g + 1],
                                scale=1.0,
                            )
                        else:
                            nc.vector.tensor_scalar_add(
                                o_tile[:], ps[:], b_tile[:, g : g + 1]
                            )
                        nc.sync.dma_start(
                            out[b, g, :, r0 * W : r0 * W + nf], o_tile[:]
                        )
    _strip_mm_tick_updates(nc)
    nc.finalize()
    return nc


_NC = None


def _prep_inputs(x, weight, bias):
    x = np.asarray(x, dtype=np.float32)
    weight = np.asarray(weight, dtype=np.float32)
    bias = np.asarray(bias, dtype=np.float32)
    xp = np.zeros((B, C_IN, HP, WPAD), dtype=BF16)
    xp[:, :, 1 : H + 1, 1 : W + 1] = x.astype(BF16)
    # wt[p, kh*3+kw, o] = weight[o, p, kh, kw]
    wt = np.ascontiguousarray(
        weight.transpose(1, 2, 3, 0).reshape(C_IN, KH * KW, C_OUT).astype(BF16)
    )
    # bz[p, g] = bias[g*128 + p]
    bz = np.ascontiguousarray(bias.reshape(NGRP, 128).T)
    return xp, wt, bz


def kernel(x, weight, bias, trace=False):
    global _NC
    xp, wt, bz = _prep_inputs(x, weight, bias)
    if _NC is None:
        _NC = _build()
    in_maps = [
        {"xp": xp[c * B_LOC : (c + 1) * B_LOC], "wt": wt, "bz": bz}
        for c in range(N_CORES)
    ]
    res = run_bass_kernel_spmd(
        _NC, in_maps, core_ids=list(range(N_CORES)), trace=trace
    )
    outs = [
        r["out"].astype(np.float32).reshape(B_LOC, C_OUT, H, W) for r in res.results
    ]
    full = np.concatenate(outs, axis=0)
    if trace:
        return full, res
    return full


# revision 26
# speedup vs baseline: 1.0008x; 1.0008x over previous
"""Conv2d 3x3 (stride 1, pad 1) as implicit GEMM on 8 Trainium2 NeuronCores.

Problem: x [32,128,56,56] f32, weight [256,128,3,3] f32, bias [256] f32
         -> out [32,256,56,56] f32.

Sharding: data-parallel over batch. Each of the 8 cores gets 4 images;
weight/bias are replicated. No collectives; outputs are concatenated on host.

Per-core kernel (implicit GEMM, bf16 operands, fp32 PSUM accumulate):
  - x is host-padded to bf16 [4,128,58,64]: 1px conv halo + row pitch padded
    56->64 so each row slice is 128B (16B-line aligned) in SBUF.
  - weight is host-rearranged to bf16 [128, 9, 256] (in-ch partitions, 3x3
    taps, out-ch free) so lhsT slices need no on-device transpose.
  - For each image, out-channel group g (2 groups of 128) and band of 8
    output rows (7 bands): accumulate 9 matmuls (one per tap) into a
    [128, 448] PSUM tile: psum += W[:, ki, g*128:...].T @ xpad[:, rows+kh, kw:kw+56]
  - bias-add + PSUM->SBUF(bf16) on scalar/vector engines alternating,
    then DMA to DRAM (bf16). Host converts the gathered output to f32.

Measured cadence ladder at N=448 (trn2 NTFF/perfetto):
  - fp32r: 210ns/MM - LDWEIGHTS-bound (fp32 LDW 187ns + ~20ns handover
    exceeds the 186.7ns stream; fp32r cannot use standalone ldweights).
  - bf16 unpadded rows: 230ns/MM (LDW 116ns hidden; +43ns suspected from
    the 8-row rhs AP blocks at 116B stride crossing 16B SBUF lines).
  - bf16 with 128B-aligned rows: this kernel. Target ~190ns/MM.
  Per-matmul tick-sem increments are stripped down to group-final matmuls
  (26.3ns/inc measured, though removing them alone did not change cadence).

Other head/tail measures:
  - Warm-up matmuls run on a memset tile (no DMA dependency) so the PE's
    HAM clock-gate (1.2 -> 2.4 GHz, fires ~2 activity windows after PE
    work starts) warms while the first input chunk is in flight.
  - DMA triggers cost ~650ns each on their (sync/scalar) queue and Tile
    rotates ~10 completion-sem slots, so the head issues exactly 10 DMAs:
    7 image-0 chunks on sync, w-group0 / bias / w-group1 bundles on
    scalar. Later images trickle in coarser chunks.
"""

import numpy as np
import ml_dtypes

import concourse.bacc as bacc
import concourse.mybir as mybir
import concourse.tile as tile
from concourse.bass_utils import run_bass_kernel_spmd

N_CORES = 8
B, C_IN, H, W = 32, 128, 56, 56
C_OUT = 256
KH = KW = 3
B_LOC = B // N_CORES          # 4 images per core
HP = H + 2                    # 58 padded rows (conv halo)
WP = W + 2                    # 58 valid padded cols
WPAD = 64                     # row pitch in SBUF: 64 bf16 = 128B, 16B-aligned
ROWS = 8                      # output rows per matmul
NCHUNK = H // ROWS            # 7 bands
NFREE = ROWS * W              # 448 = matmul free dim (fits one PSUM bank)
NGRP = C_OUT // 128           # 2 out-channel groups

MM_DT = mybir.dt.bfloat16
OUT_DT = mybir.dt.bfloat16
BF16 = ml_dtypes.bfloat16

N_WARM = 6                    # memset-fed warm-up matmuls (N=448 @ full duty)


def _strip_mm_tick_updates(nc):
    """Remove the Tile tick-sem increment from non-stop matmuls.

    Tile attaches a `sem-inc @complete` to EVERY matmul so consumers can
    wait "first v matmuls done". Matmuls complete in queue order, so it is
    sufficient for only accumulation-group-final (stop=True) matmuls to
    increment, with every wait value remapped from "v matmuls" to "k
    stop-matmuls". All Tile-emitted waits on the tick sem land on group
    boundaries (asserted below), because the only cross-engine consumers
    of matmul completion are whole-PSUM-tile readers.
    """
    import concourse.mybir as mybir
    from collections import Counter

    mms = []
    others = []
    for f in nc.m.functions:
        for blk in f.blocks:
            for inst in blk.instructions:
                if isinstance(inst, mybir.InstMatmult):
                    mms.append(inst)
                else:
                    others.append(inst)
    ids = Counter()
    for m in mms:
        si = m.sync_info
        for u in si.on_update if si else []:
            if u.update_mode == "sem-inc":
                ids[u.id] += 1
    if not ids:
        return
    tick, cnt = ids.most_common(1)[0]
    assert cnt == len(mms), (tick, cnt, len(mms))

    kept = [bool(m.stop_tensor_calc) for m in mms]
    prefix = [0]
    for k in kept:
        prefix.append(prefix[-1] + (1 if k else 0))

    def remap(v):
        assert 0 <= v <= len(mms), v
        # wait must land on a stop-matmul boundary, else ordering is lost
        assert v == 0 or kept[v - 1], f"tick wait {v} not at a group boundary"
        return prefix[v]

    for inst in mms + others:
        si = getattr(inst, "sync_info", None)
        if not si:
            continue
        for w in si.on_wait:
            if w.sync_type == "semaphore" and w.id == tick:
                assert w.wait_mode == "sem-ge-imm", w
                w.wait_value = remap(w.wait_value)
    for m, k in zip(mms, kept):
        if not k:
            si = m.sync_info
            si.on_update = [
                u
                for u in si.on_update
                if not (u.sync_type == "semaphore" and u.id == tick)
            ]


def _build():
    nc = bacc.Bacc(None, target_bir_lowering=False)
    xp = nc.dram_tensor("xp", [B_LOC, C_IN, HP, WPAD], MM_DT, kind="ExternalInput")
    # weights laid out [cin, group, tap, 128] so any (group, tap-range)
    # slice is CONTIGUOUS per partition: the previous [cin, tap, cout]
    # layout made head weight-DMAs 256B-packet sprays (measured ~4us to
    # land); contiguous slices move as 768-2304B packets.
    wt = nc.dram_tensor(
        "wt", [C_IN, NGRP, KH * KW, 128], MM_DT, kind="ExternalInput"
    )
    bz = nc.dram_tensor("bz", [128, NGRP], mybir.dt.float32, kind="ExternalInput")
    out = nc.dram_tensor(
        "out", [B_LOC, NGRP, 128, H * W], OUT_DT, kind="ExternalOutput"
    )

    with tile.TileContext(nc) as tc:
        with (
            tc.tile_pool(name="const", bufs=1) as cpool,
            tc.tile_pool(name="xin", bufs=B_LOC) as xpool,
            tc.tile_pool(name="oout", bufs=8) as opool,
            tc.tile_pool(name="psum", bufs=1, space="PSUM") as pspool,
        ):
            # PE warm-up with no DMA dependency: memset a tile, then issue
            # full-duty N=448 matmuls so the HAM clock-gate warms while the
            # first input chunk DMA is in flight.
            wu = cpool.tile([128, NFREE], MM_DT)
            nc.gpsimd.memset(wu[:], 0.0)
            wu_ps = pspool.tile([128, NFREE], mybir.dt.float32, tag="warm", bufs=1)
            for _ in range(N_WARM):
                nc.tensor.matmul(
                    wu_ps[:], wu[:, 0:128], wu[:], start=True, stop=True
                )

            w_tile = cpool.tile([C_IN, NGRP, KH * KW, 128], MM_DT)
            b_tile = cpool.tile([128, NGRP], mybir.dt.float32)
            x_tiles = [
                xpool.tile([C_IN, HP, WPAD], MM_DT, name=f"x_img{b}", tag="ximg")
                for b in range(B_LOC)
            ]

            # chunk rc of image b: band-aligned row ranges. Band rc needs
            # padded rows [rc*ROWS, rc*ROWS+ROWS+2); chunk 0 covers rows
            # 0..9, chunk rc>=1 adds rows rc*ROWS+2 .. rc*ROWS+9.
            def load_chunk(b, rc):
                lo = 0 if rc == 0 else rc * ROWS + 2
                hi = rc * ROWS + ROWS + 2
                nc.sync.dma_start(x_tiles[b][:, lo:hi], xp[b, :, lo:hi])

            # Head DMAs split across the two HWDGE trigger queues. chunk0
            # moves as two partition-halves (one per queue) so it lands in
            # half the time; group-0 weights land as taps 0-2 first so band
            # 0's first matmuls are gated only by chunk0 + those taps.
            nc.sync.dma_start(x_tiles[0][0:64, 0:10], xp[0, 0:64, 0:10])
            nc.scalar.dma_start(x_tiles[0][64:128, 0:10], xp[0, 64:128, 0:10])
            nc.scalar.dma_start(w_tile[:, 0, 0:3], wt[:, 0, 0:3])
            load_chunk(0, 1)
            nc.scalar.dma_start(w_tile[:, 0, 3:9], wt[:, 0, 3:9])
            load_chunk(0, 2)
            nc.scalar.dma_start(b_tile[:], bz[:])
            load_chunk(0, 3)
            nc.scalar.dma_start(w_tile[:, 1], wt[:, 1])
            load_chunk(0, 4)
            load_chunk(0, 5)
            load_chunk(0, 6)

            for b in range(B_LOC):
                for g in range(NGRP):
                    for rc in range(NCHUNK):
                        # trickle next image's chunks during the g=0 pass so
                        # prefetch doesn't starve this image's output DMAs
                        if g == 0 and b + 1 < B_LOC:
                            load_chunk(b + 1, rc)
                        ps = pspool.tile(
                            [128, NFREE], mybir.dt.float32, tag="ps", bufs=7
                        )
                        for ki in range(KH * KW):
                            kh, kw = divmod(ki, KW)
                            nc.tensor.matmul(
                                ps[:],
                                w_tile[:, g, ki],
                                x_tiles[b][
                                    :,
                                    rc * ROWS + kh : rc * ROWS + kh + ROWS,
                                    kw : kw + W,
                                ],
                                start=(ki == 0),
                                stop=(ki == KH * KW - 1),
                            )
                        o_tile = opool.tile(
                            [128, NFREE],
                            OUT_DT,
                            name=f"o_{b}_{g}_{rc}",
                            tag="ot",
                        )
                        last = b == B_LOC - 1 and g == NGRP - 1 and rc == NCHUNK - 1
                        if last:
                            # final band: evict in halves on both engines and
                            # DMA each half from its own trigger queue, so the
                            # post-last-matmul chain is half as long
                            half = NFREE // 2
                            nc.scalar.activation(
                                o_tile[:, 0:half],
                                ps[:, 0:half],
                                mybir.ActivationFunctionType.Identity,
                                bias=b_tile[:, g : g + 1],
                                scale=1.0,
                            )
                            nc.vector.tensor_scalar_add(
                                o_tile[:, half:], ps[:, half:], b_tile[:, g : g + 1]
                            )
                            base = rc * NFREE
                            nc.sync.dma_start(
                                out[b, g, :, base : base + half], o_tile[:, 0:half]
                            )
                            nc.scalar.dma_start(
                                out[b, g, :, base + half : base + NFREE],
                                o_tile[:, half:],
                            )
                            continue
                        # alternate eviction engine: scalar and vector can
                        # read PSUM concurrently (different banks)
                        if rc % 2 == 0:
                            nc.scalar.activation(
                                o_tile[:],
                                ps[:],
                                mybir.ActivationFunctionType.Identity,
                                bias=b_tile[:, g : g + 1],
                                scale=1.0,
                            )
                        else:
                            nc.vector.tensor_scalar_add(
                                o_tile[:], ps[:], b_tile[:, g : g + 1]
                            )
                        nc.sync.dma_start(
                            out[b, g, :, rc * NFREE : (rc + 1) * NFREE], o_tile[:]
                        )
    _strip_mm_tick_updates(nc)
    nc.finalize()
    return nc


_NC = None


def _prep_inputs(x, weight, bias):
    x = np.asarray(x, dtype=np.float32)
    weight = np.asarray(weight, dtype=np.float32)
    bias = np.asarray(bias, dtype=np.float32)
    xp = np.zeros((B, C_IN, HP, WPAD), dtype=BF16)
    xp[:, :, 1 : H + 1, 1 : W + 1] = x.astype(BF16)
    # wt[p, g, kh*3+kw, o'] = weight[g*128+o', p, kh, kw]
    wt = np.ascontiguousarray(
        weight.reshape(NGRP, 128, C_IN, KH * KW)
        .transpose(2, 0, 3, 1)
        .astype(BF16)
    )
    # bz[p, g] = bias[g*128 + p]
    bz = np.ascontiguousarray(bias.reshape(NGRP, 128).T)
    return xp, wt, bz


def kernel(x, weight, bias, trace=False):
    global _NC
    xp, wt, bz = _prep_inputs(x, weight, bias)
    if _NC is None:
        _NC = _build()
    in_maps = [
        {"xp": xp[c * B_LOC : (c + 1) * B_LOC], "wt": wt, "bz": bz}
        for c in range(N_CORES)
    ]
    res = run_bass_kernel_spmd(
        _NC, in_maps, core_ids=list(range(N_CORES)), trace=trace
    )
    outs = [
        r["out"].astype(np.float32).reshape(B_LOC, C_OUT, H, W) for r in res.results
    ]
    full = np.concatenate(outs, axis=0)
    if trace:
        return full, res
    return full
